# revision 7
# baseline (speedup 1.0000x reference)
"""Causal self-attention (B=8, T=1024, C=1024, H=16, hd=64) on 8 TRN2 cores.

Sharding: data parallel — one batch element per NeuronCore.

All matmul operands are fp16 (11-bit precision ~ f32r, full PE rate at any
moving size, half the DMA/SBUF of f32) with f32 PSUM accumulation.

Device layouts (partition dim first):
  xT     [128, 8*T]  x[b].T in 8 row-chunks of 128; loaded t-half first so
         the first projection starts ~2.5us in.
  Q^T,K^T [128, 8*T] head-pair strip hp in cols [hp*T,(hp+1)*T); a per-head
         feature permutation (evens-then-odds) folded into the weights makes
         RoPE's q1/q2 split two 16-partition blocks per 32 rows.
  RoPE: qrot = (q+b)*c2 + (swap16(q)+swap16(bs))*s2m; swap16 via DVE
        stream_shuffle, two scalar_tensor_tensor fused bias+mul, gpsimd add.
  S^T   [s,t] per head pair: lhsT = Krot^T [64,128], rhs = Qrot^T [64,<=512].
        exp on ACT out of PSUM straight to fp16; diagonal 128x128 blocks get
        a 0/1 triangle multiply (split across DVE and gpsimd).
  y^T   [t-part, d] per 128-t-block: lhsT = P~ [128s,128t] (stationary),
        rhs = [V_j | 1] [128s, 65].  Column 64 accumulates the softmax
        denominator, so normalization is one reciprocal + one broadcast
        multiply fused into the PSUM eviction; a PE transpose brings y back
        to [c,t] for the output projection.
  Loop order: t-chunk c (512) outer, head pair inner.  Projections, V, wo
        prefetch and the ch=0 output projection are interleaved as PE filler
        so the tensor engine never idles (HAM k=8/8).
"""
import numpy as np
import concourse.bass as bass
import concourse.tile as tile
import concourse.mybir as mybir
from concourse import bacc
from concourse.bass import broadcast_tensor_aps
from concourse.bass_utils import run_bass_kernel_spmd

F16 = mybir.dt.float16
F32 = mybir.dt.float32
EXP = mybir.ActivationFunctionType.Exp
ADD = mybir.AluOpType.add
MULT = mybir.AluOpType.mult

B, T, C = 8, 1024, 1024
H, HD = 16, 64
NCORES = 8


def build_program():
    nc = bacc.Bacc("TRN2", target_bir_lowering=False, debug=False)

    def din(name, shape, dt=F16):
        return nc.dram_tensor(name, shape, dt, kind="ExternalInput").ap()

    xT = din("xT", [C, T])
    wqT = din("wqT", [C, C])
    wkT = din("wkT", [C, C])
    wvT = din("wvT", [C, C])
    woT = din("woT", [C, C])
    bq = din("bq", [128, 8], F32)
    bqs = din("bqs", [128, 8], F32)
    bk = din("bk", [128, 8], F32)
    bks = din("bks", [128, 8], F32)
    bo = din("bo", [128, 8], F32)
    bv = din("bv", [1, C])
    c2 = din("c2", [128, T], F32)
    s2m = din("s2m", [128, T], F32)
    tri = din("tri", [128, 128])
    onesrow = din("onesrow", [1, 128])
    ident = din("ident", [128, 128])
    oT = nc.dram_tensor("oT", [C, T], F32, kind="ExternalOutput").ap()

    with tile.TileContext(nc) as tc:
        with (
            tc.tile_pool(name="pc", bufs=1) as pc,
            tc.tile_pool(name="pw", bufs=3) as pw,
            tc.tile_pool(name="pwv", bufs=4) as pwv,
            tc.tile_pool(name="prope", bufs=4) as prope,
            tc.tile_pool(name="ppt", bufs=4) as ppt,
            tc.tile_pool(name="pstage", bufs=2) as pstage,
            tc.tile_pool(name="prcp", bufs=2) as prcp,
            tc.tile_pool(name="posb", bufs=3) as posb,
            tc.tile_pool(name="psS", bufs=2, space="PSUM") as psS,
            tc.tile_pool(name="psY", bufs=1, space="PSUM") as psY,
            tc.tile_pool(name="psMM", bufs=2, space="PSUM") as psMM,
        ):
            ENG = [nc.sync, nc.scalar, nc.gpsimd]
            rot = [0]

            def dma(dst, src):
                ENG[rot[0] % 3].dma_start(dst, src)
                rot[0] += 1

            # ---- resident tensors / startup DMA (t-half of x first) ----
            xT_sb = pc.tile([128, 8 * T], F16, tag="xbig")
            for ct in range(4):
                nc.sync.dma_start(xT_sb[:, ct * T: ct * T + 512],
                                  xT[ct * 128:(ct + 1) * 128, 0:512])
            for ct in range(4, 8):
                nc.scalar.dma_start(xT_sb[:, ct * T: ct * T + 512],
                                    xT[ct * 128:(ct + 1) * 128, 0:512])
            btiles = {}
            for nm, ap in [("bq", bq), ("bqs", bqs), ("bk", bk), ("bks", bks),
                           ("bo", bo)]:
                t_ = pc.tile([128, 8], F32, tag=nm)
                nc.sync.dma_start(t_[:], ap)
                btiles[nm] = t_
            c2_sb = pc.tile([128, T], F32, tag="c2")
            s2_sb = pc.tile([128, T], F32, tag="s2")
            nc.sync.dma_start(c2_sb[:], c2)
            nc.scalar.dma_start(s2_sb[:], s2m)
            tri_sb = pc.tile([128, 128], F16, tag="tri")
            nc.gpsimd.dma_start(tri_sb[:], tri)
            onesrow_sb = pc.tile([1, 128], F16, tag="onesrow")
            nc.gpsimd.dma_start(onesrow_sb[:], onesrow)
            ident_sb = pc.tile([128, 128], F16, tag="ident")
            nc.gpsimd.dma_start(ident_sb[:], ident)
            bv_sb = pc.tile([1, C], F16, tag="bv")
            nc.gpsimd.dma_start(bv_sb[:], bv)
            # second t-half of x
            for ct in range(4):
                nc.sync.dma_start(xT_sb[:, ct * T + 512: ct * T + 1024],
                                  xT[ct * 128:(ct + 1) * 128, 512:1024])
            for ct in range(4, 8):
                nc.scalar.dma_start(xT_sb[:, ct * T + 512: ct * T + 1024],
                                    xT[ct * 128:(ct + 1) * 128, 512:1024])

            qrot_sb = pc.tile([128, 8 * T], F16, tag="qrot")
            krot_sb = pc.tile([128, 8 * T], F16, tag="krot")
            qrot3 = qrot_sb[:].rearrange("p (d t) -> p d t", d=8)
            krot3 = krot_sb[:].rearrange("p (d t) -> p d t", d=8)
            yt_sb = pc.tile([128, 8 * T], F16, tag="yt")
            yt3 = yt_sb[:].rearrange("p (d t) -> p d t", d=8)
            yt4 = yt_sb[:].rearrange("p (d s t) -> p d s t", d=8, s=8)
            v_sb = [pc.tile([128, 16 * 65], F16, tag=f"v{j}", name=f"v{j}")
                    for j in range(8)]
            v3 = [v_sb[j][:].rearrange("p (h j) -> p h j", j=65)
                  for j in range(8)]
            for j in range(8):
                nc.gpsimd.memset(v3[j][:, :, 64:65], 1.0)
            wo_slabs = [pc.tile([128, 8, 128], F16, tag=f"wo{e}",
                                name=f"wo{e}") for e in range(8)]

            # ---- emission helpers ----
            SHUF = list(range(16, 32)) + list(range(0, 16))

            def emit_qk_group(which, dblk, ch):
                wT, bnm, bsnm, dest = which
                wsl = pw.tile([128, 8, 128], F16, tag="w",
                              name=f"w{bnm}{dblk}_{ch}")
                dma(wsl[:], wT[:, dblk * 128:(dblk + 1) * 128].rearrange(
                    "(ct p) m -> p ct m", p=128))
                ps = psMM.tile([128, 512], F32, tag="mm",
                               name=f"p{bnm}{dblk}_{ch}")
                for ct in range(8):
                    nc.tensor.matmul(
                        ps[:], wsl[:, ct, :],
                        xT_sb[:, ct * T + ch * 512: ct * T + ch * 512 + 512],
                        start=(ct == 0), stop=(ct == 7))
                qsw = prope.tile([128, 512], F32, tag="qsw",
                                 name=f"qsw{bnm}{dblk}_{ch}")
                nc.vector.stream_shuffle(qsw[:], ps[:], mask=SHUF)
                dsl = dest[:, dblk, ch * 512:(ch + 1) * 512]
                nc.vector.scalar_tensor_tensor(
                    dsl, ps[:], btiles[bnm][:, dblk:dblk + 1],
                    c2_sb[:, ch * 512: ch * 512 + 512], op0=ADD, op1=MULT)
                qsb = prope.tile([128, 512], F16, tag="qsb",
                                 name=f"qsb{bnm}{dblk}_{ch}")
                nc.vector.scalar_tensor_tensor(
                    qsb[:], qsw[:], btiles[bsnm][:, dblk:dblk + 1],
                    s2_sb[:, ch * 512: ch * 512 + 512], op0=ADD, op1=MULT)
                nc.gpsimd.tensor_add(dsl, dsl, qsb[:])

            wv_slabs = {}

            def emit_v_group(ch, sblk):
                if (ch, 0) not in wv_slabs:
                    wv_r = wvT[:, ch * 512:(ch + 1) * 512].rearrange(
                        "(ct p) m -> p ct m", p=128)
                    for half in range(2):
                        vsl = pwv.tile([128, 4, 512], F16, tag="wv",
                                       name=f"wv{ch}_{half}")
                        dma(vsl[:], wv_r[:, half * 4: half * 4 + 4, :])
                        wv_slabs[(ch, half)] = vsl
                ps = psMM.tile([128, 512], F32, tag="mm", name=f"pv{ch}_{sblk}")
                for ct in range(8):
                    vsl = wv_slabs[(ch, ct // 4)]
                    nc.tensor.matmul(
                        ps[:],
                        xT_sb[:, ct * T + sblk * 128: ct * T + sblk * 128 + 128],
                        vsl[:, ct % 4, :],
                        start=(ct == 0), stop=False)
                nc.tensor.matmul(
                    ps[:], onesrow_sb[:], bv_sb[:, ch * 512:(ch + 1) * 512],
                    start=False, stop=True)
                nc.scalar.copy(v3[sblk][:, 8 * ch: 8 * ch + 8, 0:64], ps[:])

            def emit_wo_dma(eblk):
                dma(wo_slabs[eblk][:],
                    woT[:, eblk * 128:(eblk + 1) * 128].rearrange(
                        "(ct p) m -> p ct m", p=128))

            def emit_outproj(eblk, ch):
                ps = psMM.tile([128, 512], F32, tag="mm", name=f"o{eblk}_{ch}")
                for dt in range(8):
                    nc.tensor.matmul(
                        ps[:], wo_slabs[eblk][:, dt, :],
                        yt3[:, dt, ch * 512:(ch + 1) * 512],
                        start=(dt == 0), stop=(dt == 7))
                osb = posb.tile([128, 512], F32, tag="osb")
                nc.vector.tensor_scalar_add(osb[:], ps[:],
                                            btiles["bo"][:, eblk:eblk + 1])
                dma(oT[eblk * 128:(eblk + 1) * 128, ch * 512:(ch + 1) * 512],
                    osb[:])

            QSPEC = (wqT, "bq", "bqs", qrot3)
            KSPEC = (wkT, "bk", "bks", krot3)

            def attention(c, hp, fillers):
                njs = 4 * c + 4
                psY_t = psY.tile([128, 1024], F32, tag="y",
                                 name=f"y{c}_{hp}")
                psY4 = psY_t[:].rearrange("p (s hi t) -> p s hi t",
                                          s=4, hi=2, t=128)
                for j in range(njs):
                    nst = 128 * (j - 4 * c) if j >= 4 * c else 0
                    ps_s = psS.tile([128, 1024], F32, tag="s",
                                    name=f"s{c}_{hp}_{j}")
                    ps_s3 = ps_s[:].rearrange("p (g t) -> p g t", g=2)
                    for hi in range(2):
                        r0 = 64 * hi
                        nc.tensor.matmul(
                            ps_s3[:, hi, nst:512],
                            krot_sb[r0:r0 + 64,
                                    hp * T + j * 128: hp * T + j * 128 + 128],
                            qrot_sb[r0:r0 + 64,
                                    hp * T + c * 512 + nst: hp * T + c * 512 + 512],
                            start=True, stop=True)
                    p_t = ppt.tile([128, 2, 512], F16, tag="pt",
                                   name=f"pt{c}_{hp}_{j}")
                    nc.scalar.activation(p_t[:, :, nst:512],
                                         ps_s3[:, :, nst:512],
                                         EXP, scale=0.125)
                    if j >= 4 * c:
                        nc.vector.tensor_mul(
                            p_t[:, 0, nst:nst + 128],
                            p_t[:, 0, nst:nst + 128], tri_sb[:])
                        nc.gpsimd.tensor_mul(
                            p_t[:, 1, nst:nst + 128],
                            p_t[:, 1, nst:nst + 128], tri_sb[:])
                    if fillers:
                        f = fillers.pop(0)
                        if f is not None:
                            f()
                    # one start=True per PSUM bank: start clears the whole
                    # bank's has_written bits; later start=False writes then
                    # overwrite-first-touch / accumulate-after per element.
                    for tsub in range(max(0, j - 4 * c), 4):
                        for hi in range(2):
                            nc.tensor.matmul(
                                psY4[:, tsub, hi, 0:65],
                                p_t[:, hi, tsub * 128: tsub * 128 + 128],
                                v3[j][:, 2 * hp + hi, :],
                                start=(j == 0 and hi == 0
                                       and tsub in (0, 2)),
                                stop=(j == 4 * c + tsub and hi == 1
                                      and tsub in (1, 3)),
                                skip_group_check=True)
                # ---- eviction: recip + fused normalize + transpose ----
                rcp = prcp.tile([128, 4, 2, 1], F32, tag="rcp")
                nc.vector.reciprocal(rcp[:], psY4[:, :, :, 64:65])
                stage = pstage.tile([128, 4, 2, 64], F16, tag="st")
                for hi in range(2):
                    in0 = psY4[:, :, hi, 0:64]
                    in0b, in1b = broadcast_tensor_aps(in0, rcp[:, :, hi, :])
                    nc.vector.tensor_mul(stage[:, :, hi, :], in0b, in1b)
                if fillers:
                    f = fillers.pop(0)
                    if f is not None:
                        f()
                ps_tr = psMM.tile([128, 8, 128], F16, tag="mm",
                                  name=f"tr{c}_{hp}")
                for tsub in range(4):
                    nc.tensor.transpose(ps_tr[:, tsub, :], stage[:, tsub],
                                        ident_sb[:])
                nc.vector.tensor_copy(yt4[:, hp, c * 4: c * 4 + 4, :],
                                      ps_tr[:, 0:4, :])

            # ---- schedule ----
            Q = lambda d, ch: (lambda: emit_qk_group(QSPEC, d, ch))
            K = lambda d, ch: (lambda: emit_qk_group(KSPEC, d, ch))
            V = lambda ch, s: (lambda: emit_v_group(ch, s))
            WO = lambda e: (lambda: emit_wo_dma(e))
            OP = lambda e, ch: (lambda: emit_outproj(e, ch))

            emit_qk_group(QSPEC, 0, 0)
            emit_qk_group(KSPEC, 0, 0)
            emit_v_group(0, 0)
            emit_v_group(0, 1)

            c0_fill = [
                V(0, 2), V(0, 3), Q(1, 0), K(1, 0), Q(2, 0),
                K(2, 0), V(1, 0), V(1, 1), Q(3, 0), K(3, 0),
                V(1, 2), V(1, 3), Q(4, 0), K(4, 0), Q(5, 0),
                K(5, 0), Q(6, 0), K(6, 0), Q(7, 0), K(7, 0),
                Q(0, 1), K(0, 1), V(0, 4), V(0, 5), V(0, 6),
                V(0, 7), WO(0), WO(1), WO(2), WO(3),
                WO(4), WO(5), None, None, None,
                None, None, None, None, None,
            ]
            c1_fill = [None] * 72
            for slot, item in [
                (0, WO(6)), (1, Q(1, 1)), (2, K(1, 1)), (3, WO(7)),
                (4, OP(0, 0)), (7, Q(2, 1)), (8, K(2, 1)), (10, OP(1, 0)),
                (12, V(1, 4)), (15, Q(3, 1)), (16, K(3, 1)), (18, OP(2, 0)),
                (20, V(1, 5)), (23, Q(4, 1)), (24, K(4, 1)), (27, OP(3, 0)),
                (28, V(1, 6)), (31, Q(5, 1)), (32, K(5, 1)), (33, V(1, 7)),
                (36, OP(4, 0)), (40, Q(6, 1)), (41, K(6, 1)), (45, OP(5, 0)),
                (49, Q(7, 1)), (50, K(7, 1)), (54, OP(6, 0)), (63, OP(7, 0)),
            ]:
                c1_fill[slot] = item

            for hp in range(8):
                attention(0, hp, c0_fill)
            for hp in range(8):
                attention(1, hp, c1_fill)
            for e in range(8):
                emit_outproj(e, 1)

    nc.compile()
    return nc


def prep_inputs(x, wq, bq, wk, bk, wv, bv, wo, bo):
    """Host-side prep: per-head feature permutation, transposes, RoPE tables."""
    f16, f32 = np.float16, np.float32
    perm = np.concatenate([
        np.arange(0, 32, 2), np.arange(1, 32, 2),
        np.arange(32, 64, 2), np.arange(33, 64, 2),
    ])
    pidx = np.concatenate([h * HD + perm for h in range(H)])

    wq_p, bq_p = wq[pidx], bq[pidx]
    wk_p, bk_p = wk[pidx], bk[pidx]
    swap = lambda v: np.ascontiguousarray(
        v.reshape(2 * H, 2, 16)[:, ::-1].reshape(-1))
    bt = lambda v: np.ascontiguousarray(v.reshape(8, 128).T, dtype=f32)

    inv_freq = (1.0 / (10000.0 ** (np.arange(0, HD, 2, dtype=np.float64) / HD)))
    th = np.outer(np.arange(T, dtype=np.float64), inv_freq)
    cosT = np.cos(th).T.astype(f32)
    sinT = np.sin(th).T.astype(f32)
    c64 = np.concatenate([cosT[0:16], cosT[0:16], cosT[16:32], cosT[16:32]])
    s64 = np.concatenate([-sinT[0:16], sinT[0:16], -sinT[16:32], sinT[16:32]])
    c2 = np.ascontiguousarray(np.tile(c64, (2, 1)), dtype=f32)
    s2m = np.ascontiguousarray(np.tile(s64, (2, 1)), dtype=f32)

    shared = {
        "wqT": np.ascontiguousarray(wq_p.T, dtype=f16),
        "wkT": np.ascontiguousarray(wk_p.T, dtype=f16),
        "wvT": np.ascontiguousarray(wv.T, dtype=f16),
        "woT": np.ascontiguousarray(wo.T, dtype=f16),
        "bq": bt(bq_p), "bqs": bt(swap(bq_p)),
        "bk": bt(bk_p), "bks": bt(swap(bk_p)),
        "bo": bt(bo),
        "bv": np.ascontiguousarray(bv[None, :], dtype=f16),
        "c2": c2, "s2m": s2m,
        "tri": np.triu(np.ones((128, 128), dtype=f16)),
        "onesrow": np.ones((1, 128), dtype=f16),
        "ident": np.eye(128, dtype=f16),
    }
    in_maps = []
    for b in range(B):
        m = dict(shared)
        m["xT"] = np.ascontiguousarray(np.asarray(x[b]).T, dtype=f16)
        in_maps.append(m)
    return in_maps


_nc_cache = None


def run(inputs, trace=False, trace_kwargs=None):
    global _nc_cache
    if _nc_cache is None:
        _nc_cache = build_program()
    in_maps = prep_inputs(
        np.asarray(inputs["x"], dtype=np.float32),
        *[np.asarray(inputs[k], dtype=np.float32)
          for k in ["wq", "bq", "wk", "bk", "wv", "bv", "wo", "bo"]])
    res = run_bass_kernel_spmd(_nc_cache, in_maps, list(range(NCORES)),
                               trace=trace, **(trace_kwargs or {}))
    out = np.stack([np.ascontiguousarray(res.results[b]["oT"].T)
                    for b in range(B)]).astype(np.float32)
    return out, res


def kernel(**inputs):
    out, _ = run(inputs, trace=False)
    return out


# revision 8
# speedup vs baseline: 1.1295x; 1.1295x over previous
"""Causal self-attention (B=8, T=1024, C=1024, H=16, hd=64) on 8 TRN2 cores.

Sharding: data parallel — one batch element per NeuronCore.

All matmul operands are fp16 (11-bit precision ~ f32r, full PE rate at any
moving size, half the DMA/SBUF of f32) with f32 PSUM accumulation.

Device layouts (partition dim first):
  xT     [128, 8*T]  x[b].T in 8 row-chunks of 128; t-half 0 loaded first so
         the first projection starts a few us in.
  wqS/wkS/wvS/woS [128, 8192]  weights pre-rearranged on host into slab
         layout so every weight DMA is long contiguous lines (256B-line
         gather DMAs otherwise flood the queues with tiny packets).
  Q^T,K^T [128, 8*T] head-pair strip hp in cols [hp*T,(hp+1)*T); a per-head
         feature permutation (evens-then-odds) folded into the weights makes
         RoPE's q1/q2 split two 16-partition blocks per 32 rows.
  RoPE: qrot = (q+b)*c2 + (swap16(q)+swap16(bs))*s2m via DVE stream_shuffle,
        two scalar_tensor_tensor fused bias+mul, gpsimd add.
  S^T   [s,t] per head pair: lhsT = Krot^T [64,128], rhs = Qrot^T [64,<=512].
        exp on ACT straight out of PSUM to fp16; diagonal 128x128 blocks get
        a 0/1 triangle multiply (split across DVE and gpsimd).
  y^T   [t-part, d] per 128-t-block: lhsT = P~ [128s,128t] (stationary),
        rhs = [V_j | 1] [128s, 65].  Column 64 accumulates the softmax
        denominator, so normalization is one reciprocal + one broadcast
        multiply fused into the PSUM eviction; a PE transpose brings y back
        to [c,t] for the output projection.  PSUM gotcha: start=True clears
        has_written for the WHOLE bank, so exactly one start per bank and
        everything else start=False (first-touch overwrites, then adds).
  Loop order: t-chunk c (512) outer, head pair inner.  Projections, V, wo
        prefetch and the ch=0 output projection are interleaved as PE filler
        (two half-unit pops per j-step) so the tensor engine never idles.
"""
import numpy as np
import concourse.bass as bass
import concourse.tile as tile
import concourse.mybir as mybir
from concourse import bacc
from concourse.bass import broadcast_tensor_aps
from concourse.bass_utils import run_bass_kernel_spmd

F16 = mybir.dt.float16
F32 = mybir.dt.float32
EXP = mybir.ActivationFunctionType.Exp
ADD = mybir.AluOpType.add
MULT = mybir.AluOpType.mult

B, T, C = 8, 1024, 1024
H, HD = 16, 64
NCORES = 8


def build_program():
    nc = bacc.Bacc("TRN2", target_bir_lowering=False, debug=False)

    def din(name, shape, dt=F16):
        return nc.dram_tensor(name, shape, dt, kind="ExternalInput").ap()

    xT = din("xT", [C, T])
    wqS = din("wqS", [128, 8192])
    wkS = din("wkS", [128, 8192])
    wvS = din("wvS", [128, 8192])
    woS = din("woS", [128, 8192])
    bq = din("bq", [128, 8], F32)
    bqs = din("bqs", [128, 8], F32)
    bk = din("bk", [128, 8], F32)
    bks = din("bks", [128, 8], F32)
    bo = din("bo", [128, 8], F32)
    bv = din("bv", [1, C])
    c2 = din("c2", [128, T], F32)
    s2m = din("s2m", [128, T], F32)
    tri = din("tri", [128, 128])
    onesrow = din("onesrow", [1, 128])
    ident = din("ident", [128, 128])
    oT = nc.dram_tensor("oT", [C, T], F32, kind="ExternalOutput").ap()

    with tile.TileContext(nc) as tc:
        with (
            tc.tile_pool(name="pc", bufs=1) as pc,
            tc.tile_pool(name="pw", bufs=3) as pw,
            tc.tile_pool(name="pwv", bufs=4) as pwv,
            tc.tile_pool(name="prope", bufs=4) as prope,
            tc.tile_pool(name="ppt", bufs=4) as ppt,
            tc.tile_pool(name="pstage", bufs=2) as pstage,
            tc.tile_pool(name="prcp", bufs=2) as prcp,
            tc.tile_pool(name="posb", bufs=3) as posb,
            tc.tile_pool(name="psS", bufs=2, space="PSUM") as psS,
            tc.tile_pool(name="psY", bufs=1, space="PSUM") as psY,
            tc.tile_pool(name="psMM", bufs=2, space="PSUM") as psMM,
        ):
            ENG = [nc.gpsimd, nc.sync, nc.scalar]
            rot = [0]

            def dma(dst, src):
                ENG[rot[0] % 3].dma_start(dst, src)
                rot[0] += 1

            # ---- resident tensors / startup DMA (t-half of x first) ----
            xT_sb = pc.tile([128, 8 * T], F16, tag="xbig")
            for ct in range(4):
                nc.sync.dma_start(xT_sb[:, ct * T: ct * T + 512],
                                  xT[ct * 128:(ct + 1) * 128, 0:512])
            for ct in range(4, 8):
                nc.scalar.dma_start(xT_sb[:, ct * T: ct * T + 512],
                                    xT[ct * 128:(ct + 1) * 128, 0:512])
            btiles = {}
            for nm, ap in [("bq", bq), ("bqs", bqs), ("bk", bk), ("bks", bks),
                           ("bo", bo)]:
                t_ = pc.tile([128, 8], F32, tag=nm)
                nc.sync.dma_start(t_[:], ap)
                btiles[nm] = t_
            c2_sb = pc.tile([128, T], F32, tag="c2")
            s2_sb = pc.tile([128, T], F32, tag="s2")
            nc.sync.dma_start(c2_sb[:], c2)
            nc.scalar.dma_start(s2_sb[:], s2m)
            tri_sb = pc.tile([128, 128], F16, tag="tri")
            nc.gpsimd.dma_start(tri_sb[:], tri)
            onesrow_sb = pc.tile([1, 128], F16, tag="onesrow")
            nc.gpsimd.dma_start(onesrow_sb[:], onesrow)
            ident_sb = pc.tile([128, 128], F16, tag="ident")
            nc.gpsimd.dma_start(ident_sb[:], ident)
            bv_sb = pc.tile([1, C], F16, tag="bv")
            nc.gpsimd.dma_start(bv_sb[:], bv)

            qrot_sb = pc.tile([128, 8 * T], F16, tag="qrot")
            krot_sb = pc.tile([128, 8 * T], F16, tag="krot")
            qrot3 = qrot_sb[:].rearrange("p (d t) -> p d t", d=8)
            krot3 = krot_sb[:].rearrange("p (d t) -> p d t", d=8)
            yt_sb = pc.tile([128, 8 * T], F16, tag="yt")
            yt3 = yt_sb[:].rearrange("p (d t) -> p d t", d=8)
            yt4 = yt_sb[:].rearrange("p (d s t) -> p d s t", d=8, s=8)
            v_sb = [pc.tile([128, 16 * 65], F16, tag=f"v{j}", name=f"v{j}")
                    for j in range(8)]
            v3 = [v_sb[j][:].rearrange("p (h j) -> p h j", j=65)
                  for j in range(8)]
            for j in range(8):
                nc.gpsimd.memset(v3[j][:, :, 64:65], 1.0)
            wo_slabs = [pc.tile([128, 8, 128], F16, tag=f"wo{e}",
                                name=f"wo{e}") for e in range(8)]

            # ---- emission helpers (units split into two 4-matmul halves
            # so fillers can be popped twice per attention j-step) ----
            SHUF = list(range(16, 32)) + list(range(0, 16))
            hstate = {}

            def emit_qk_half(which, dblk, ch, half):
                wS, bnm, bsnm, dest = which
                key = ("qk", bnm, dblk, ch)
                if half == 0:
                    wsl = pw.tile([128, 8, 128], F16, tag="w",
                                  name=f"w{bnm}{dblk}_{ch}")
                    dma(wsl[:],
                        wS[:, dblk * 1024:(dblk + 1) * 1024].rearrange(
                            "p (ct m) -> p ct m", m=128))
                    ps = psMM.tile([128, 512], F32, tag="mm",
                                   name=f"p{bnm}{dblk}_{ch}")
                    for ct in range(4):
                        nc.tensor.matmul(
                            ps[:], wsl[:, ct, :],
                            xT_sb[:, ct * T + ch * 512: ct * T + ch * 512 + 512],
                            start=(ct == 0), stop=False)
                    hstate[key] = (wsl, ps)
                    return
                wsl, ps = hstate.pop(key)
                for ct in range(4, 8):
                    nc.tensor.matmul(
                        ps[:], wsl[:, ct, :],
                        xT_sb[:, ct * T + ch * 512: ct * T + ch * 512 + 512],
                        start=False, stop=(ct == 7))
                qsw = prope.tile([128, 512], F32, tag="qsw",
                                 name=f"qsw{bnm}{dblk}_{ch}")
                nc.vector.stream_shuffle(qsw[:], ps[:], mask=SHUF)
                dsl = dest[:, dblk, ch * 512:(ch + 1) * 512]
                nc.vector.scalar_tensor_tensor(
                    dsl, ps[:], btiles[bnm][:, dblk:dblk + 1],
                    c2_sb[:, ch * 512: ch * 512 + 512], op0=ADD, op1=MULT)
                qsb = prope.tile([128, 512], F16, tag="qsb",
                                 name=f"qsb{bnm}{dblk}_{ch}")
                nc.vector.scalar_tensor_tensor(
                    qsb[:], qsw[:], btiles[bsnm][:, dblk:dblk + 1],
                    s2_sb[:, ch * 512: ch * 512 + 512], op0=ADD, op1=MULT)
                nc.gpsimd.tensor_add(dsl, dsl, qsb[:])

            wv_slabs = {}

            def emit_v_half(ch, sblk, half):
                key = ("v", ch, sblk)
                if half == 0:
                    if (ch, 0) not in wv_slabs:
                        for hs in range(2):
                            vsl = pwv.tile([128, 4, 512], F16, tag="wv",
                                           name=f"wv{ch}_{hs}")
                            dma(vsl[:],
                                wvS[:, ch * 4096 + hs * 2048:
                                    ch * 4096 + hs * 2048 + 2048].rearrange(
                                    "p (q m) -> p q m", m=512))
                            wv_slabs[(ch, hs)] = vsl
                    ps = psMM.tile([128, 512], F32, tag="mm",
                                   name=f"pv{ch}_{sblk}")
                    vsl = wv_slabs[(ch, 0)]
                    for ct in range(4):
                        nc.tensor.matmul(
                            ps[:],
                            xT_sb[:, ct * T + sblk * 128: ct * T + sblk * 128 + 128],
                            vsl[:, ct, :], start=(ct == 0), stop=False)
                    hstate[key] = ps
                    return
                ps = hstate.pop(key)
                vsl = wv_slabs[(ch, 1)]
                for ct in range(4, 8):
                    nc.tensor.matmul(
                        ps[:],
                        xT_sb[:, ct * T + sblk * 128: ct * T + sblk * 128 + 128],
                        vsl[:, ct - 4, :], start=False, stop=False)
                nc.tensor.matmul(
                    ps[:], onesrow_sb[:], bv_sb[:, ch * 512:(ch + 1) * 512],
                    start=False, stop=True)
                nc.scalar.copy(v3[sblk][:, 8 * ch: 8 * ch + 8, 0:64], ps[:])

            def emit_wo_dma(eblk):
                dma(wo_slabs[eblk][:],
                    woS[:, eblk * 1024:(eblk + 1) * 1024].rearrange(
                        "p (ct m) -> p ct m", m=128))

            def emit_op_half(eblk, ch, half):
                key = ("op", eblk, ch)
                if half == 0:
                    ps = psMM.tile([128, 512], F32, tag="mm",
                                   name=f"o{eblk}_{ch}")
                    for dt in range(4):
                        nc.tensor.matmul(
                            ps[:], wo_slabs[eblk][:, dt, :],
                            yt3[:, dt, ch * 512:(ch + 1) * 512],
                            start=(dt == 0), stop=False)
                    hstate[key] = ps
                    return
                ps = hstate.pop(key)
                for dt in range(4, 8):
                    nc.tensor.matmul(
                        ps[:], wo_slabs[eblk][:, dt, :],
                        yt3[:, dt, ch * 512:(ch + 1) * 512],
                        start=False, stop=(dt == 7))
                osb = posb.tile([128, 512], F32, tag="osb")
                nc.vector.tensor_scalar_add(osb[:], ps[:],
                                            btiles["bo"][:, eblk:eblk + 1])
                dma(oT[eblk * 128:(eblk + 1) * 128, ch * 512:(ch + 1) * 512],
                    osb[:])

            QSPEC = (wqS, "bq", "bqs", qrot3)
            KSPEC = (wkS, "bk", "bks", krot3)

            def pops(fillers, n=2):
                for _ in range(n):
                    if fillers:
                        f = fillers.pop(0)
                        if f is not None:
                            f()

            def attention(c, hp, fillers):
                njs = 4 * c + 4
                psY_t = psY.tile([128, 1024], F32, tag="y",
                                 name=f"y{c}_{hp}")
                psY4 = psY_t[:].rearrange("p (s hi t) -> p s hi t",
                                          s=4, hi=2, t=128)
                for j in range(njs):
                    nst = 128 * (j - 4 * c) if j >= 4 * c else 0
                    ps_s = psS.tile([128, 1024], F32, tag="s",
                                    name=f"s{c}_{hp}_{j}")
                    ps_s3 = ps_s[:].rearrange("p (g t) -> p g t", g=2)
                    for hi in range(2):
                        r0 = 64 * hi
                        nc.tensor.matmul(
                            ps_s3[:, hi, nst:512],
                            krot_sb[r0:r0 + 64,
                                    hp * T + j * 128: hp * T + j * 128 + 128],
                            qrot_sb[r0:r0 + 64,
                                    hp * T + c * 512 + nst: hp * T + c * 512 + 512],
                            start=True, stop=True)
                    p_t = ppt.tile([128, 2, 512], F16, tag="pt",
                                   name=f"pt{c}_{hp}_{j}")
                    nc.scalar.activation(p_t[:, :, nst:512],
                                         ps_s3[:, :, nst:512],
                                         EXP, scale=0.125)
                    if j >= 4 * c:
                        nc.vector.tensor_mul(
                            p_t[:, 0, nst:nst + 128],
                            p_t[:, 0, nst:nst + 128], tri_sb[:])
                        nc.gpsimd.tensor_mul(
                            p_t[:, 1, nst:nst + 128],
                            p_t[:, 1, nst:nst + 128], tri_sb[:])
                    pops(fillers)
                    # one start=True per PSUM bank (start clears the whole
                    # bank's has_written bits; first-touch then overwrites,
                    # accumulate after).
                    for tsub in range(max(0, j - 4 * c), 4):
                        for hi in range(2):
                            nc.tensor.matmul(
                                psY4[:, tsub, hi, 0:65],
                                p_t[:, hi, tsub * 128: tsub * 128 + 128],
                                v3[j][:, 2 * hp + hi, :],
                                start=(j == 0 and hi == 0
                                       and tsub in (0, 2)),
                                stop=(j == 4 * c + tsub and hi == 1
                                      and tsub in (1, 3)),
                                skip_group_check=True)
                # ---- eviction: recip + fused normalize + transpose ----
                rcp = prcp.tile([128, 4, 2, 1], F32, tag="rcp")
                nc.vector.reciprocal(rcp[:], psY4[:, :, :, 64:65])
                stage = pstage.tile([128, 4, 2, 64], F16, tag="st")
                for hi in range(2):
                    in0 = psY4[:, :, hi, 0:64]
                    in0b, in1b = broadcast_tensor_aps(in0, rcp[:, :, hi, :])
                    nc.vector.tensor_mul(stage[:, :, hi, :], in0b, in1b)
                pops(fillers)
                ps_tr = psS.tile([128, 8, 128], F16, tag="s",
                                 name=f"tr{c}_{hp}")
                for tsub in range(4):
                    nc.tensor.transpose(ps_tr[:, tsub, :], stage[:, tsub],
                                        ident_sb[:])
                nc.vector.tensor_copy(yt4[:, hp, c * 4: c * 4 + 4, :],
                                      ps_tr[:, 0:4, :])

            # ---- schedule ----
            Qh = lambda d, ch, h: (lambda: emit_qk_half(QSPEC, d, ch, h))
            Kh = lambda d, ch, h: (lambda: emit_qk_half(KSPEC, d, ch, h))
            Vh = lambda ch, s, h: (lambda: emit_v_half(ch, s, h))
            WO = lambda e: (lambda: emit_wo_dma(e))
            OPh = lambda e, ch, h: (lambda: emit_op_half(e, ch, h))

            for h in range(2):
                emit_qk_half(QSPEC, 0, 0, h)
            for h in range(2):
                emit_qk_half(KSPEC, 0, 0, h)
            for h in range(2):
                emit_v_half(0, 0, h)
            for h in range(2):
                emit_v_half(0, 1, h)
            # second t-half of x (after upfront so its DMAs queue behind)
            for ct in range(4):
                nc.sync.dma_start(xT_sb[:, ct * T + 512: ct * T + 1024],
                                  xT[ct * 128:(ct + 1) * 128, 512:1024])
            for ct in range(4, 8):
                nc.scalar.dma_start(xT_sb[:, ct * T + 512: ct * T + 1024],
                                    xT[ct * 128:(ct + 1) * 128, 512:1024])

            def qk_pair(d, ch):
                return [Qh(d, ch, 0), Qh(d, ch, 1), Kh(d, ch, 0), Kh(d, ch, 1)]

            def v_pair(ch, s):
                return [Vh(ch, s, 0), Vh(ch, s, 1)]

            c0_fill = (
                v_pair(0, 2) + v_pair(0, 3) + qk_pair(1, 0)[:4]
                + v_pair(0, 4)                                    # hp0 (10)
                + qk_pair(2, 0) + v_pair(0, 5) + v_pair(0, 6)
                + v_pair(0, 7)                                    # hp1 (10)
                + qk_pair(3, 0) + v_pair(1, 0) + v_pair(1, 1)
                + [WO(0), WO(1)]                                  # hp2 (10)
                + qk_pair(4, 0) + v_pair(1, 2) + v_pair(1, 3)
                + [WO(2), WO(3)]                                  # hp3 (10)
                + qk_pair(5, 0) + v_pair(1, 4) + v_pair(1, 5)
                + [WO(4), WO(5)]                                  # hp4 (10)
                + qk_pair(6, 0) + v_pair(1, 6) + v_pair(1, 7)
                + [WO(6), WO(7)]                                  # hp5 (10)
                + qk_pair(7, 0) + qk_pair(0, 1) + [None, None]    # hp6 (10)
                + qk_pair(1, 1) + qk_pair(2, 1) + [None, None]    # hp7 (10)
            )
            c1_items = (
                [OPh(0, 0, 0), OPh(0, 0, 1)] + qk_pair(3, 1)
                + [OPh(1, 0, 0), OPh(1, 0, 1)] + qk_pair(4, 1)
                + [OPh(2, 0, 0), OPh(2, 0, 1), OPh(3, 0, 0), OPh(3, 0, 1)]
                + qk_pair(5, 1)
                + [OPh(4, 0, 0), OPh(4, 0, 1)] + qk_pair(6, 1)
                + [OPh(5, 0, 0), OPh(5, 0, 1)] + qk_pair(7, 1)
                + [OPh(6, 0, 0), OPh(6, 0, 1), OPh(7, 0, 0), OPh(7, 0, 1)]
            )
            c1_fill = [None] * 144
            for i, item in enumerate(c1_items):
                c1_fill[4 * i] = item

            for hp in range(8):
                attention(0, hp, c0_fill)
            for hp in range(8):
                attention(1, hp, c1_fill)
            for e in range(8):
                emit_op_half(e, 1, 0)
                emit_op_half(e, 1, 1)

    nc.compile()
    return nc


def prep_inputs(x, wq, bq, wk, bk, wv, bv, wo, bo):
    """Host-side prep: per-head feature permutation, slab layouts, RoPE."""
    f16, f32 = np.float16, np.float32
    perm = np.concatenate([
        np.arange(0, 32, 2), np.arange(1, 32, 2),
        np.arange(32, 64, 2), np.arange(33, 64, 2),
    ])
    pidx = np.concatenate([h * HD + perm for h in range(H)])

    wq_p, bq_p = wq[pidx], bq[pidx]
    wk_p, bk_p = wk[pidx], bk[pidx]
    swap = lambda v: np.ascontiguousarray(
        v.reshape(2 * H, 2, 16)[:, ::-1].reshape(-1))
    bt = lambda v: np.ascontiguousarray(v.reshape(8, 128).T, dtype=f32)
    # slab layout: [p, dblk, ct, m] flattened to [128, 8192]; a slab's DMA
    # is then one [128, 1024] contiguous read (2KB lines).
    slab8 = lambda wT: np.ascontiguousarray(
        wT.reshape(8, 128, 8, 128).transpose(1, 2, 0, 3).reshape(128, 8192),
        dtype=f16)
    slabv = lambda wT: np.ascontiguousarray(
        wT.reshape(8, 128, 2, 512).transpose(1, 2, 0, 3).reshape(128, 8192),
        dtype=f16)

    inv_freq = (1.0 / (10000.0 ** (np.arange(0, HD, 2, dtype=np.float64) / HD)))
    th = np.outer(np.arange(T, dtype=np.float64), inv_freq)
    cosT = np.cos(th).T.astype(f32)
    sinT = np.sin(th).T.astype(f32)
    c64 = np.concatenate([cosT[0:16], cosT[0:16], cosT[16:32], cosT[16:32]])
    s64 = np.concatenate([-sinT[0:16], sinT[0:16], -sinT[16:32], sinT[16:32]])
    c2 = np.ascontiguousarray(np.tile(c64, (2, 1)), dtype=f32)
    s2m = np.ascontiguousarray(np.tile(s64, (2, 1)), dtype=f32)

    shared = {
        "wqS": slab8(wq_p.T), "wkS": slab8(wk_p.T),
        "wvS": slabv(wv.T), "woS": slab8(wo.T),
        "bq": bt(bq_p), "bqs": bt(swap(bq_p)),
        "bk": bt(bk_p), "bks": bt(swap(bk_p)),
        "bo": bt(bo),
        "bv": np.ascontiguousarray(bv[None, :], dtype=f16),
        "c2": c2, "s2m": s2m,
        "tri": np.triu(np.ones((128, 128), dtype=f16)),
        "onesrow": np.ones((1, 128), dtype=f16),
        "ident": np.eye(128, dtype=f16),
    }
    in_maps = []
    for b in range(B):
        m = dict(shared)
        m["xT"] = np.ascontiguousarray(np.asarray(x[b]).T, dtype=f16)
        in_maps.append(m)
    return in_maps


_nc_cache = None


def run(inputs, trace=False, trace_kwargs=None):
    global _nc_cache
    if _nc_cache is None:
        _nc_cache = build_program()
    in_maps = prep_inputs(
        np.asarray(inputs["x"], dtype=np.float32),
        *[np.asarray(inputs[k], dtype=np.float32)
          for k in ["wq", "bq", "wk", "bk", "wv", "bv", "wo", "bo"]])
    res = run_bass_kernel_spmd(_nc_cache, in_maps, list(range(NCORES)),
                               trace=trace, **(trace_kwargs or {}))
    out = np.stack([np.ascontiguousarray(res.results[b]["oT"].T)
                    for b in range(B)]).astype(np.float32)
    return out, res


def kernel(**inputs):
    out, _ = run(inputs, trace=False)
    return out


# revision 17
# speedup vs baseline: 1.1810x; 1.0455x over previous
"""Causal self-attention (B=8, T=1024, C=1024, H=16, hd=64) on 8 TRN2 cores.

Sharding: data parallel — one batch element per NeuronCore.

All matmul operands are fp16 (11-bit precision ~ f32r, full PE rate at any
moving size, half the DMA/SBUF of f32) with f32 PSUM accumulation.

Device layouts (partition dim first):
  xT     [128, 8*T]  x[b].T in 8 row-chunks of 128; t-half 0 loaded first so
         the first projection starts a few us in.
  wqS/wkS/wvS/woS [128, 8192]  weights pre-rearranged on host into slab
         layout so every weight DMA is long contiguous lines (256B-line
         gather DMAs otherwise flood the queues with tiny packets).
  Q^T,K^T [128, 8*T] head-pair strip hp in cols [hp*T,(hp+1)*T); a per-head
         feature permutation (evens-then-odds) folded into the weights makes
         RoPE's q1/q2 split two 16-partition blocks per 32 rows.
  RoPE: qrot = (q+b)*c2 + (swap16(q)+swap16(bs))*s2m via DVE stream_shuffle,
        two scalar_tensor_tensor fused bias+mul, gpsimd add.
  S^T   [s,t] per head pair: lhsT = Krot^T [64,128], rhs = Qrot^T [64,<=512].
        exp on ACT straight out of PSUM to fp16; diagonal 128x128 blocks get
        a 0/1 triangle multiply (split across DVE and gpsimd).
  y^T   [t-part, d] per 128-t-block: lhsT = P~ [128s,128t] (stationary),
        rhs = [V_j | 1] [128s, 65].  Column 64 accumulates the softmax
        denominator, so normalization is one reciprocal + one broadcast
        multiply fused into the PSUM eviction; a PE transpose brings y back
        to [c,t] for the output projection.  PSUM gotcha: start=True clears
        has_written for the WHOLE bank, so exactly one start per bank and
        everything else start=False (first-touch overwrites, then adds).
  Loop order: t-chunk c (512) outer, head pair inner.  Projections, V, wo
        prefetch and the ch=0 output projection are interleaved as PE filler
        (two half-unit pops per j-step) so the tensor engine never idles.
"""
import numpy as np
import concourse.bass as bass
import concourse.tile as tile
import concourse.mybir as mybir
from concourse import bacc
from concourse.bass import broadcast_tensor_aps
from concourse.bass_utils import run_bass_kernel_spmd

F16 = mybir.dt.float16
F32 = mybir.dt.float32
EXP = mybir.ActivationFunctionType.Exp
ADD = mybir.AluOpType.add
MULT = mybir.AluOpType.mult

B, T, C = 8, 1024, 1024
H, HD = 16, 64
NCORES = 8


def build_program():
    nc = bacc.Bacc("TRN2", target_bir_lowering=False, debug=False)

    def din(name, shape, dt=F16):
        return nc.dram_tensor(name, shape, dt, kind="ExternalInput").ap()

    xT = din("xT", [C, T])
    wqS = din("wqS", [128, 8192])
    wkS = din("wkS", [128, 8192])
    wvS = din("wvS", [128, 8192])
    woS = din("woS", [128, 8192])
    ball = din("ball", [128, 40], F32)   # bq|bqs|bk|bks|bo
    c2 = din("c2", [128, T], F32)
    s2m = din("s2m", [128, T], F32)
    misc = din("misc", [128, 1280])      # tri | ident | bv in row 0
    oT = nc.dram_tensor("oT", [C, T], F32, kind="ExternalOutput").ap()

    with tile.TileContext(nc) as tc:
        with (
            tc.tile_pool(name="pc", bufs=1) as pc,
            tc.tile_pool(name="pw", bufs=3) as pw,
            tc.tile_pool(name="pwv", bufs=4) as pwv,
            tc.tile_pool(name="prope", bufs=4) as prope,
            tc.tile_pool(name="ppt", bufs=4) as ppt,
            tc.tile_pool(name="pstage", bufs=2) as pstage,
            tc.tile_pool(name="prcp", bufs=2) as prcp,
            tc.tile_pool(name="posb", bufs=3) as posb,
            tc.tile_pool(name="psS", bufs=2, space="PSUM") as psS,
            tc.tile_pool(name="psY", bufs=1, space="PSUM") as psY,
            tc.tile_pool(name="psMM", bufs=2, space="PSUM") as psMM,
        ):
            ENG = [nc.gpsimd, nc.sync, nc.scalar]
            rot = [0]

            def dma(dst, src):
                ENG[rot[0] % 3].dma_start(dst, src)
                rot[0] += 1

            # ---- resident tensors / startup DMA (t-half of x first) ----
            xT_sb = pc.tile([128, 8 * T], F16, tag="xbig")
            xT_r = xT.rearrange("(ct p) t -> p ct t", p=128)
            xv = xT_sb[:].rearrange("p (ct t) -> p ct t", t=T)
            nc.sync.dma_start(xv[:, 0:4, 0:512], xT_r[:, 0:4, 0:512])
            nc.scalar.dma_start(xv[:, 4:8, 0:512], xT_r[:, 4:8, 0:512])
            ball_sb = pc.tile([128, 40], F32, tag="ball")
            nc.sync.dma_start(ball_sb[:], ball)
            btiles = {nm: ball_sb[:, 8 * i: 8 * i + 8]
                      for i, nm in enumerate(["bq", "bqs", "bk", "bks", "bo"])}
            c2_sb = pc.tile([128, T], F32, tag="c2")
            s2_sb = pc.tile([128, T], F32, tag="s2")
            nc.sync.dma_start(c2_sb[:], c2)
            nc.scalar.dma_start(s2_sb[:], s2m)
            misc_sb = pc.tile([128, 1280], F16, tag="misc")
            tri_sb = misc_sb[:, 0:128]
            ident_sb = misc_sb[:, 128:256]
            onesrow_sb = misc_sb[0:1, 0:128]     # row 0 of triu == ones
            bv_sb = misc_sb[0:1, 256:1280]

            qrot_sb = pc.tile([128, 8 * T], F16, tag="qrot")
            krot_sb = pc.tile([128, 8 * T], F16, tag="krot")
            qrot3 = qrot_sb[:].rearrange("p (d t) -> p d t", d=8)
            krot3 = krot_sb[:].rearrange("p (d t) -> p d t", d=8)
            yt_sb = pc.tile([128, 8 * T], F16, tag="yt")
            yt3 = yt_sb[:].rearrange("p (d t) -> p d t", d=8)
            yt4 = yt_sb[:].rearrange("p (d s t) -> p d s t", d=8, s=8)
            v_sb = [pc.tile([128, 16 * 65], F16, tag=f"v{j}", name=f"v{j}")
                    for j in range(8)]
            v3 = [v_sb[j][:].rearrange("p (h j) -> p h j", j=65)
                  for j in range(8)]
            wo_slabs = [pc.tile([128, 8, 128], F16, tag=f"wo{e}",
                                name=f"wo{e}") for e in range(8)]

            # ---- emission helpers (units split into two 4-matmul halves
            # so fillers can be popped twice per attention j-step) ----
            SHUF = list(range(16, 32)) + list(range(0, 16))
            hstate = {}

            def emit_qk_half(which, dblk, ch, half):
                wS, bnm, bsnm, dest = which
                key = ("qk", bnm, dblk, ch)
                if half == 0:
                    wsl = pw.tile([128, 8, 128], F16, tag="w",
                                  name=f"w{bnm}{dblk}_{ch}")
                    dma(wsl[:],
                        wS[:, dblk * 1024:(dblk + 1) * 1024].rearrange(
                            "p (ct m) -> p ct m", m=128))
                    ps = psMM.tile([128, 512], F32, tag="mm",
                                   name=f"p{bnm}{dblk}_{ch}")
                    for ct in range(4):
                        nc.tensor.matmul(
                            ps[:], wsl[:, ct, :],
                            xT_sb[:, ct * T + ch * 512: ct * T + ch * 512 + 512],
                            start=(ct == 0), stop=False)
                    hstate[key] = (wsl, ps)
                    return
                wsl, ps = hstate.pop(key)
                for ct in range(4, 8):
                    nc.tensor.matmul(
                        ps[:], wsl[:, ct, :],
                        xT_sb[:, ct * T + ch * 512: ct * T + ch * 512 + 512],
                        start=False, stop=(ct == 7))
                qsw = prope.tile([128, 512], F32, tag="qsw",
                                 name=f"qsw{bnm}{dblk}_{ch}")
                nc.vector.stream_shuffle(qsw[:], ps[:], mask=SHUF)
                dsl = dest[:, dblk, ch * 512:(ch + 1) * 512]
                nc.vector.scalar_tensor_tensor(
                    dsl, ps[:], btiles[bnm][:, dblk:dblk + 1],
                    c2_sb[:, ch * 512: ch * 512 + 512], op0=ADD, op1=MULT)
                qsb = prope.tile([128, 512], F16, tag="qsb",
                                 name=f"qsb{bnm}{dblk}_{ch}")
                nc.vector.scalar_tensor_tensor(
                    qsb[:], qsw[:], btiles[bsnm][:, dblk:dblk + 1],
                    s2_sb[:, ch * 512: ch * 512 + 512], op0=ADD, op1=MULT)
                nc.gpsimd.tensor_add(dsl, dsl, qsb[:])

            wv_slabs = {}

            def emit_v_half(ch, sblk, half):
                key = ("v", ch, sblk)
                if half == 0:
                    if (ch, 0) not in wv_slabs:
                        for hs in range(2):
                            vsl = pwv.tile([128, 4, 512], F16, tag="wv",
                                           name=f"wv{ch}_{hs}")
                            dma(vsl[:],
                                wvS[:, ch * 4096 + hs * 2048:
                                    ch * 4096 + hs * 2048 + 2048].rearrange(
                                    "p (q m) -> p q m", m=512))
                            wv_slabs[(ch, hs)] = vsl
                    ps = psMM.tile([128, 512], F32, tag="mm",
                                   name=f"pv{ch}_{sblk}")
                    vsl = wv_slabs[(ch, 0)]
                    for ct in range(4):
                        nc.tensor.matmul(
                            ps[:],
                            xT_sb[:, ct * T + sblk * 128: ct * T + sblk * 128 + 128],
                            vsl[:, ct, :], start=(ct == 0), stop=False)
                    hstate[key] = ps
                    return
                ps = hstate.pop(key)
                vsl = wv_slabs[(ch, 1)]
                for ct in range(4, 8):
                    nc.tensor.matmul(
                        ps[:],
                        xT_sb[:, ct * T + sblk * 128: ct * T + sblk * 128 + 128],
                        vsl[:, ct - 4, :], start=False, stop=False)
                nc.tensor.matmul(
                    ps[:], onesrow_sb, bv_sb[:, ch * 512:(ch + 1) * 512],
                    start=False, stop=True)
                nc.scalar.copy(v3[sblk][:, 8 * ch: 8 * ch + 8, 0:64], ps[:])

            def emit_wo_dma(eblk):
                dma(wo_slabs[eblk][:],
                    woS[:, eblk * 1024:(eblk + 1) * 1024].rearrange(
                        "p (ct m) -> p ct m", m=128))

            def emit_op_half(eblk, ch, half):
                key = ("op", eblk, ch)
                if half == 0:
                    ps = psMM.tile([128, 512], F32, tag="mm",
                                   name=f"o{eblk}_{ch}")
                    for dt in range(4):
                        nc.tensor.matmul(
                            ps[:], wo_slabs[eblk][:, dt, :],
                            yt3[:, dt, ch * 512:(ch + 1) * 512],
                            start=(dt == 0), stop=False)
                    hstate[key] = ps
                    return
                ps = hstate.pop(key)
                for dt in range(4, 8):
                    nc.tensor.matmul(
                        ps[:], wo_slabs[eblk][:, dt, :],
                        yt3[:, dt, ch * 512:(ch + 1) * 512],
                        start=False, stop=(dt == 7))
                osb = posb.tile([128, 512], F32, tag="osb")
                nc.vector.tensor_scalar_add(osb[:], ps[:],
                                            btiles["bo"][:, eblk:eblk + 1])
                nc.sync.dma_start(
                    oT[eblk * 128:(eblk + 1) * 128, ch * 512:(ch + 1) * 512],
                    osb[:])

            QSPEC = (wqS, "bq", "bqs", qrot3)
            KSPEC = (wkS, "bk", "bks", krot3)

            def pops(fillers, n=2):
                for _ in range(n):
                    if fillers:
                        f = fillers.pop(0)
                        if f is not None:
                            f()

            def attention(c, hp, fillers):
                njs = 4 * c + 4
                psY_t = psY.tile([128, 1024], F32, tag="y",
                                 name=f"y{c}_{hp}")
                psY4 = psY_t[:].rearrange("p (s hi t) -> p s hi t",
                                          s=4, hi=2, t=128)
                for j in range(njs):
                    nst = 128 * (j - 4 * c) if j >= 4 * c else 0
                    ps_s = psS.tile([128, 1024], F32, tag="s",
                                    name=f"s{c}_{hp}_{j}")
                    ps_s3 = ps_s[:].rearrange("p (g t) -> p g t", g=2)
                    for hi in range(2):
                        r0 = 64 * hi
                        nc.tensor.matmul(
                            ps_s3[:, hi, nst:512],
                            krot_sb[r0:r0 + 64,
                                    hp * T + j * 128: hp * T + j * 128 + 128],
                            qrot_sb[r0:r0 + 64,
                                    hp * T + c * 512 + nst: hp * T + c * 512 + 512],
                            start=True, stop=True)
                    p_t = ppt.tile([128, 2, 512], F16, tag="pt",
                                   name=f"pt{c}_{hp}_{j}")
                    nc.scalar.activation(p_t[:, :, nst:512],
                                         ps_s3[:, :, nst:512],
                                         EXP, scale=0.125)
                    if j >= 4 * c:
                        nc.vector.tensor_mul(
                            p_t[:, 0, nst:nst + 128],
                            p_t[:, 0, nst:nst + 128], tri_sb)
                        nc.gpsimd.tensor_mul(
                            p_t[:, 1, nst:nst + 128],
                            p_t[:, 1, nst:nst + 128], tri_sb)
                    pops(fillers)
                    # one start=True per PSUM bank (start clears the whole
                    # bank's has_written bits; first-touch then overwrites,
                    # accumulate after).
                    for tsub in range(max(0, j - 4 * c), 4):
                        for hi in range(2):
                            nc.tensor.matmul(
                                psY4[:, tsub, hi, 0:65],
                                p_t[:, hi, tsub * 128: tsub * 128 + 128],
                                v3[j][:, 2 * hp + hi, :],
                                start=(j == 0 and hi == 0
                                       and tsub in (0, 2)),
                                stop=(j == 4 * c + tsub and hi == 1
                                      and tsub in (1, 3)),
                                skip_group_check=True)
                # ---- eviction: recip + fused normalize + transpose ----
                rcp = prcp.tile([128, 4, 2, 1], F32, tag="rcp")
                nc.vector.reciprocal(rcp[:], psY4[:, :, :, 64:65])
                stage = pstage.tile([128, 4, 2, 64], F16, tag="st")
                for hi in range(2):
                    in0 = psY4[:, :, hi, 0:64]
                    in0b, in1b = broadcast_tensor_aps(in0, rcp[:, :, hi, :])
                    nc.vector.tensor_mul(stage[:, :, hi, :], in0b, in1b)
                pops(fillers)
                ps_tr = psS.tile([128, 8, 128], F16, tag="s",
                                 name=f"tr{c}_{hp}")
                for tsub in range(4):
                    nc.tensor.transpose(ps_tr[:, tsub, :], stage[:, tsub],
                                        ident_sb)
                nc.vector.tensor_copy(yt4[:, hp, c * 4: c * 4 + 4, :],
                                      ps_tr[:, 0:4, :])

            # ---- schedule ----
            Qh = lambda d, ch, h: (lambda: emit_qk_half(QSPEC, d, ch, h))
            Kh = lambda d, ch, h: (lambda: emit_qk_half(KSPEC, d, ch, h))
            Vh = lambda ch, s, h: (lambda: emit_v_half(ch, s, h))
            WO = lambda e: (lambda: emit_wo_dma(e))
            OPh = lambda e, ch, h: (lambda: emit_op_half(e, ch, h))

            emit_qk_half(QSPEC, 0, 0, 0)
            nc.gpsimd.dma_start(misc_sb[:], misc)
            emit_qk_half(QSPEC, 0, 0, 1)
            for h in range(2):
                emit_qk_half(KSPEC, 0, 0, h)
            for h in range(2):
                emit_v_half(0, 0, h)
            for h in range(2):
                emit_v_half(0, 1, h)
            for j in range(8):
                nc.gpsimd.memset(v3[j][:, :, 64:65], 1.0)
            # second t-half of x (after upfront so its DMAs queue behind)
            nc.sync.dma_start(xv[:, 0:4, 512:1024], xT_r[:, 0:4, 512:1024])
            nc.scalar.dma_start(xv[:, 4:8, 512:1024], xT_r[:, 4:8, 512:1024])

            def qk_pair(d, ch):
                return [Qh(d, ch, 0), Qh(d, ch, 1), Kh(d, ch, 0), Kh(d, ch, 1)]

            def v_pair(ch, s):
                return [Vh(ch, s, 0), Vh(ch, s, 1)]

            c0_fill = (
                v_pair(0, 2) + v_pair(0, 3) + qk_pair(1, 0)
                + v_pair(0, 4)                                    # hp0 (10)
                + qk_pair(2, 0) + v_pair(0, 5) + v_pair(0, 6)
                + v_pair(0, 7)                                    # hp1 (10)
                + qk_pair(3, 0) + v_pair(1, 0) + v_pair(1, 1)
                + [WO(0), WO(1)]                                  # hp2 (10)
                + qk_pair(4, 0) + v_pair(1, 2) + v_pair(1, 3)
                + [WO(2), WO(3)]                                  # hp3 (10)
                + qk_pair(5, 0)
                + [WO(4), WO(5), WO(6), WO(7), None, None]        # hp4 (10)
                + qk_pair(6, 0) + qk_pair(0, 1) + [None, None]    # hp5 (10)
                + qk_pair(7, 0) + qk_pair(1, 1) + [None, None]    # hp6 (10)
                + qk_pair(2, 1) + [None] * 6                      # hp7 (10)
            )
            # c1: sparse fillers, density weighted toward late hps; V(1,s4-7)
            # land in hp1-2 (consumed by hp4's j=4..7).
            c1_sched = [
                (2, OPh(0, 0, 0)), (8, OPh(0, 0, 1)), (14, Qh(3, 1, 0)),
                (20, Qh(3, 1, 1)), (23, Kh(3, 1, 0)), (26, Kh(3, 1, 1)),
                (29, Vh(1, 4, 0)), (32, Vh(1, 4, 1)), (35, Vh(1, 5, 0)),
                (38, Vh(1, 5, 1)), (41, Vh(1, 6, 0)), (44, Vh(1, 6, 1)),
                (47, Vh(1, 7, 0)), (50, Vh(1, 7, 1)), (53, OPh(1, 0, 0)),
                (56, OPh(1, 0, 1)), (59, Qh(4, 1, 0)), (62, Qh(4, 1, 1)),
                (65, Kh(4, 1, 0)), (68, Kh(4, 1, 1)), (71, OPh(2, 0, 0)),
                (73, OPh(2, 0, 1)), (76, Qh(5, 1, 0)), (79, Qh(5, 1, 1)),
                (82, Kh(5, 1, 0)), (85, Kh(5, 1, 1)), (88, OPh(3, 0, 0)),
                (91, OPh(3, 0, 1)), (94, Qh(6, 1, 0)), (97, Qh(6, 1, 1)),
                (100, Kh(6, 1, 0)), (103, Kh(6, 1, 1)), (106, OPh(4, 0, 0)),
                (109, OPh(4, 0, 1)), (112, Qh(7, 1, 0)), (115, Qh(7, 1, 1)),
                (118, Kh(7, 1, 0)), (121, Kh(7, 1, 1)), (124, OPh(5, 0, 0)),
                (127, OPh(5, 0, 1)), (130, OPh(6, 0, 0)), (133, OPh(6, 0, 1)),
                (136, OPh(7, 0, 0)), (139, OPh(7, 0, 1)),
            ]
            c1_fill = [None] * 144
            for pos, item in c1_sched:
                c1_fill[pos] = item

            for hp in range(8):
                attention(0, hp, c0_fill)
            for hp in range(8):
                attention(1, hp, c1_fill)
            for e in range(8):
                emit_op_half(e, 1, 0)
                emit_op_half(e, 1, 1)

    nc.compile()
    return nc


def prep_inputs(x, wq, bq, wk, bk, wv, bv, wo, bo):
    """Host-side prep: per-head feature permutation, slab layouts, RoPE."""
    f16, f32 = np.float16, np.float32
    perm = np.concatenate([
        np.arange(0, 32, 2), np.arange(1, 32, 2),
        np.arange(32, 64, 2), np.arange(33, 64, 2),
    ])
    pidx = np.concatenate([h * HD + perm for h in range(H)])

    wq_p, bq_p = wq[pidx], bq[pidx]
    wk_p, bk_p = wk[pidx], bk[pidx]
    swap = lambda v: np.ascontiguousarray(
        v.reshape(2 * H, 2, 16)[:, ::-1].reshape(-1))
    bt = lambda v: np.ascontiguousarray(v.reshape(8, 128).T, dtype=f32)
    # slab layout: [p, dblk, ct, m] flattened to [128, 8192]; a slab's DMA
    # is then one [128, 1024] contiguous read (2KB lines).
    slab8 = lambda wT: np.ascontiguousarray(
        wT.reshape(8, 128, 8, 128).transpose(1, 2, 0, 3).reshape(128, 8192),
        dtype=f16)
    slabv = lambda wT: np.ascontiguousarray(
        wT.reshape(8, 128, 2, 512).transpose(1, 2, 0, 3).reshape(128, 8192),
        dtype=f16)

    inv_freq = (1.0 / (10000.0 ** (np.arange(0, HD, 2, dtype=np.float64) / HD)))
    th = np.outer(np.arange(T, dtype=np.float64), inv_freq)
    cosT = np.cos(th).T.astype(f32)
    sinT = np.sin(th).T.astype(f32)
    c64 = np.concatenate([cosT[0:16], cosT[0:16], cosT[16:32], cosT[16:32]])
    s64 = np.concatenate([-sinT[0:16], sinT[0:16], -sinT[16:32], sinT[16:32]])
    c2 = np.ascontiguousarray(np.tile(c64, (2, 1)), dtype=f32)
    s2m = np.ascontiguousarray(np.tile(s64, (2, 1)), dtype=f32)

    ball = np.concatenate(
        [bt(bq_p), bt(swap(bq_p)), bt(bk_p), bt(swap(bk_p)), bt(bo)], axis=1)
    misc = np.zeros((128, 1280), dtype=f16)
    misc[:, 0:128] = np.triu(np.ones((128, 128), dtype=f16))
    misc[:, 128:256] = np.eye(128, dtype=f16)
    misc[0, 256:1280] = bv.astype(f16)
    shared = {
        "wqS": slab8(wq_p.T), "wkS": slab8(wk_p.T),
        "wvS": slabv(wv.T), "woS": slab8(wo.T),
        "ball": np.ascontiguousarray(ball),
        "c2": c2, "s2m": s2m,
        "misc": misc,
    }
    in_maps = []
    for b in range(B):
        m = dict(shared)
        m["xT"] = np.ascontiguousarray(np.asarray(x[b]).T, dtype=f16)
        in_maps.append(m)
    return in_maps


_nc_cache = None


def run(inputs, trace=False, trace_kwargs=None):
    global _nc_cache
    if _nc_cache is None:
        _nc_cache = build_program()
    in_maps = prep_inputs(
        np.asarray(inputs["x"], dtype=np.float32),
        *[np.asarray(inputs[k], dtype=np.float32)
          for k in ["wq", "bq", "wk", "bk", "wv", "bv", "wo", "bo"]])
    res = run_bass_kernel_spmd(_nc_cache, in_maps, list(range(NCORES)),
                               trace=trace, **(trace_kwargs or {}))
    out = np.stack([np.ascontiguousarray(res.results[b]["oT"].T)
                    for b in range(B)]).astype(np.float32)
    return out, res


def kernel(**inputs):
    out, _ = run(inputs, trace=False)
    return out
